# revision 25
# baseline (speedup 1.0000x reference)
"""BPR loss with pre-sampled negatives, data-parallel over batch on 8 NeuronCores.

Reference computation (B=16, T=100, V=50000, S=4):
    pos = output[b, t, labels[b, t]]
    neg = output[b, t, neg_ids[b, t, s]]
    ls  = log_sigmoid(pos - neg)
    loss = mean_b( -sum_t(mask_t * sum_s ls) / x_lens[b] )

Only 5 of the 50000 vocab logits per (b, t) are touched, so each core
gathers its 1000 needed elements with indirect DMAs instead of streaming
its 40MB shard.  Sharding (host) is pure relayout: the batch is split 2
users/core; labels+neg_ids are packed into one [BL, T, 5] index tensor;
x_lens values (as f32) are appended to the flat logit shard at elements
FLAT + b*V.

Per core (2 users):
  - one SWDGE DMA loads the ids transposed to [t, (b k)] layout,
    accumulated on top of an iota-built row base (b*T + t)*V
  - 10 indirect DMAs ([128, 1] each — the HW consumes one gather index
    per partition channel per instruction) fetch the pos/neg logits
  - x_lens broadcasts to all partitions via one step-0 DMA
  - sigmoid(pos - neg) on ACT per user (overlapping the other user's
    gathers), then sum_s ln(sig) = ln(prod_s sig): pair-products on DVE
    and a single Ln — exactly two activation-table loads
  - weights (t < x_lens[b]) / x_lens[b] on DVE
  - partition-axis sum via ones-vector matmul on the otherwise idle PE
  - core writes one scalar partial; host sums the 8 partials.
"""

import os
import sys

import numpy as np

for _p in ("/opt/trn_rl_repo", "/root/.axon_site/_ro/trn_rl_repo"):
    if os.path.isdir(_p) and _p not in sys.path:
        sys.path.insert(0, _p)

import concourse.bass as bass
import concourse.mybir as mybir
from concourse import bacc
from concourse.tile import TileContext

B, T, V, S = 16, 100, 50000, 4
NCORES = 8
BL = B // NCORES          # users per core
K = S + 1                 # ids per (b, t): label + S negs
W = BL * K                # gather columns
P = 128
FLAT = BL * T * V         # flat size of the per-core logit shard
DLEN = FLAT + V + 1       # + tail holding x_lens (f32 values) at FLAT + b*V

f32 = mybir.dt.float32
i32 = mybir.dt.int32


def build_bass() -> bass.Bass:
    nc = bacc.Bacc("TRN2", target_bir_lowering=False, debug=False)
    data = nc.declare_dram_parameter("data", [DLEN], f32, isOutput=False)
    ids_in = nc.declare_dram_parameter("ids_in", [BL, T, K], i32, isOutput=False)
    partial = nc.declare_dram_parameter("partial", [1, 1], f32, isOutput=True)

    with TileContext(nc) as tc:
        with (
            tc.tile_pool(name="pool", bufs=1) as pool,
            tc.tile_pool(name="psum", bufs=1, space="PSUM") as psum,
        ):
            # --- gather indices: (b*T + t)*V + id, partition = t.  iota
            # steps must fit int16, so the *V happens on DVE; the ids land
            # on top of the row base via SWDGE accumulate-during-DMA ---
            btb = pool.tile([P, W], i32)
            nc.gpsimd.iota(
                btb[:], pattern=[[T, BL], [0, K]], base=0, channel_multiplier=1
            )
            idx = pool.tile([P, W], i32)
            nc.vector.tensor_scalar_mul(out=idx[:], in0=btb[:], scalar1=V)
            nc.gpsimd.dma_start(
                out=idx[:T, :].rearrange("t (b k) -> t b k", b=BL),
                in_=ids_in[:].transpose([1, 0, 2]),
                accum_op=mybir.AluOpType.add,
            )

            # --- per-element gathers ([128, 1] per instruction) + per-user
            # compute, so user 0's ACT chain overlaps user 1's gathers ---
            vals = pool.tile([P, W], f32)
            nc.vector.memset(vals[:], 0.0)
            # one shared bounds register instead of a RegisterMove per gather
            bc_reg = nc.gpsimd.to_reg(DLEN - 1)
            d = pool.tile([P, BL * S], f32)
            sg = pool.tile([P, BL * S], f32)
            m2 = pool.tile([P, BL * 2], f32)
            prod = pool.tile([P, BL], f32)
            for b in range(BL):
                for c in range(b * K, (b + 1) * K):
                    nc.gpsimd.indirect_dma_start(
                        out=vals[:, c : c + 1],
                        out_offset=None,
                        in_=data[:, None],
                        in_offset=bass.IndirectOffsetOnAxis(
                            ap=idx[:, c : c + 1], axis=0
                        ),
                        bounds_check=bc_reg,
                        oob_is_err=False,
                    )
                bs = slice(b * S, (b + 1) * S)
                nc.vector.tensor_tensor(
                    out=d[:, bs],
                    in0=vals[:, b * K : b * K + 1].to_broadcast([P, S]),
                    in1=vals[:, b * K + 1 : (b + 1) * K],
                    op=mybir.AluOpType.subtract,
                )
                # sigmoid per user so the first one hides under the other
                # user's gathers (and only one sigmoid-table load total)
                nc.scalar.activation(
                    out=sg[:, bs],
                    in_=d[:, bs],
                    func=mybir.ActivationFunctionType.Sigmoid,
                )
                # sum_s ln(sig) == ln(prod_s sig): pair-multiply on DVE so a
                # single Ln instruction (one ln-table load) finishes the chain
                nc.vector.tensor_tensor(
                    out=m2[:, 2 * b : 2 * b + 2],
                    in0=sg[:, b * S : b * S + 2],
                    in1=sg[:, b * S + 2 : b * S + 4],
                    op=mybir.AluOpType.mult,
                )
                nc.vector.tensor_tensor(
                    out=prod[:, b : b + 1],
                    in0=m2[:, 2 * b : 2 * b + 1],
                    in1=m2[:, 2 * b + 1 : 2 * b + 2],
                    op=mybir.AluOpType.mult,
                )
            # --- x_lens broadcast: one step-0 DMA replicates the two f32
            # values at data[FLAT + b*V] to every partition ---
            xlf = pool.tile([P, BL], f32)
            nc.gpsimd.dma_start(
                out=xlf[:],
                in_=data[FLAT : FLAT + V + 1 : V][None, :].to_broadcast([P, BL]),
            )
            inv = pool.tile([P, BL], f32)
            nc.vector.reciprocal(out=inv[:], in_=xlf[:])
            tio = pool.tile([P, 1], i32)
            nc.gpsimd.iota(tio[:], pattern=[[0, 1]], base=0, channel_multiplier=1)
            tf = pool.tile([P, 1], f32)
            nc.vector.tensor_copy(out=tf[:], in_=tio[:])
            mask = pool.tile([P, BL], f32)
            nc.vector.tensor_tensor(
                out=mask[:],
                in0=tf[:].to_broadcast([P, BL]),
                in1=xlf[:],
                op=mybir.AluOpType.is_lt,
            )
            w = pool.tile([P, BL], f32)
            nc.vector.tensor_tensor(
                out=w[:], in0=mask[:], in1=inv[:], op=mybir.AluOpType.mult
            )

            spsum = pool.tile([P, BL], f32)
            nc.scalar.activation(
                out=spsum[:], in_=prod[:], func=mybir.ActivationFunctionType.Ln
            )

            # stage through DVE so the mult needs only one cross-engine wait
            spsum2 = pool.tile([P, BL], f32)
            nc.vector.tensor_copy(out=spsum2[:], in_=spsum[:])
            sc = pool.tile([P, BL], f32)
            nc.vector.tensor_tensor(
                out=sc[:], in0=spsum2[:], in1=w[:], op=mybir.AluOpType.mult
            )

            # --- partition-axis sum via ones matmul on PE, scale by -1/B ---
            ones = pool.tile([P, 1], f32)
            nc.vector.memset(ones[:], 1.0)
            acc = psum.tile([1, BL], f32, space="PSUM")
            nc.tensor.matmul(
                out=acc[:], lhsT=ones[:], rhs=sc[:], start=True, stop=True
            )
            res = pool.tile([1, 1], f32)
            nc.vector.reduce_sum(out=res[:], in_=acc[:], axis=mybir.AxisListType.X)
            res2 = pool.tile([1, 1], f32)
            nc.scalar.mul(out=res2[:], in_=res[:], mul=-1.0 / B)
            nc.sync.dma_start(out=partial[:], in_=res2[:])

    nc.compile()
    return nc


def make_in_maps(output, labels, x_lens, neg_ids):
    out_np = np.asarray(output, dtype=np.float32)
    lab_np = np.asarray(labels, dtype=np.int32)
    neg_np = np.asarray(neg_ids, dtype=np.int32)
    xl_np = np.asarray(x_lens, dtype=np.int32)
    ids_np = np.ascontiguousarray(
        np.concatenate([lab_np[:, :, None], neg_np], axis=2)
    )  # [B, T, K]
    in_maps = []
    for i in range(NCORES):
        sl = slice(i * BL, (i + 1) * BL)
        data = np.empty(DLEN, dtype=np.float32)
        data[:FLAT] = out_np[sl].reshape(-1)
        data[FLAT:] = 0.0
        for b in range(BL):
            data[FLAT + b * V] = np.float32(xl_np[i * BL + b])
        in_maps.append({"data": data, "ids_in": ids_np[sl]})
    return in_maps


def kernel(output, labels, x_lens, uids, neg_ids):
    from concourse.bass_utils import run_bass_kernel_spmd

    nc = build_bass()
    in_maps = make_in_maps(output, labels, x_lens, neg_ids)
    results = run_bass_kernel_spmd(nc, in_maps, list(range(NCORES))).results
    partials = np.stack(
        [np.asarray(results[i]["partial"]).reshape(()) for i in range(NCORES)]
    )
    return partials.sum(dtype=np.float32).reshape(1)


# revision 26
# speedup vs baseline: 1.0193x; 1.0193x over previous
"""BPR loss with pre-sampled negatives, data-parallel over batch on 8 NeuronCores.

Reference computation (B=16, T=100, V=50000, S=4):
    pos = output[b, t, labels[b, t]]
    neg = output[b, t, neg_ids[b, t, s]]
    ls  = log_sigmoid(pos - neg)
    loss = mean_b( -sum_t(mask_t * sum_s ls) / x_lens[b] )

Only 5 of the 50000 vocab logits per (b, t) are touched, so each core
gathers its 1000 needed elements with indirect DMAs instead of streaming
its 40MB shard.  Sharding (host) is pure relayout: the batch is split 2
users/core; labels+neg_ids are packed into one [BL, T, 5] index tensor;
x_lens values (as f32) are appended to the flat logit shard at elements
FLAT + b*V.

Per core (2 users):
  - one SWDGE DMA loads the ids transposed to [t, (b k)] layout,
    accumulated on top of an iota-built row base (b*T + t)*V
  - 10 indirect DMAs ([128, 1] each — the HW consumes one gather index
    per partition channel per instruction) fetch the pos/neg logits
  - x_lens broadcasts to all partitions via one step-0 DMA
  - sigmoid(pos - neg) on ACT per user (overlapping the other user's
    gathers), then sum_s ln(sig) = ln(prod_s sig): pair-products on DVE
    and a single Ln — exactly two activation-table loads
  - weights (t < x_lens[b]) / x_lens[b] on DVE
  - partition-axis sum via ones-vector matmul on the otherwise idle PE
  - core writes one scalar partial; host sums the 8 partials.
"""

import os
import sys

import numpy as np

for _p in ("/opt/trn_rl_repo", "/root/.axon_site/_ro/trn_rl_repo"):
    if os.path.isdir(_p) and _p not in sys.path:
        sys.path.insert(0, _p)

import concourse.bass as bass
import concourse.mybir as mybir
from concourse import bacc
from concourse.tile import TileContext

B, T, V, S = 16, 100, 50000, 4
NCORES = 8
BL = B // NCORES          # users per core
K = S + 1                 # ids per (b, t): label + S negs
W = BL * K                # gather columns
P = 128
FLAT = BL * T * V         # flat size of the per-core logit shard
DLEN = FLAT + V + 1       # + tail holding x_lens (f32 values) at FLAT + b*V

f32 = mybir.dt.float32
i32 = mybir.dt.int32


def build_bass() -> bass.Bass:
    nc = bacc.Bacc("TRN2", target_bir_lowering=False, debug=False)
    data = nc.declare_dram_parameter("data", [DLEN], f32, isOutput=False)
    ids_in = nc.declare_dram_parameter("ids_in", [T, W], i32, isOutput=False)
    partial = nc.declare_dram_parameter("partial", [1, 1], f32, isOutput=True)

    with TileContext(nc) as tc:
        with (
            tc.tile_pool(name="pool", bufs=1) as pool,
            tc.tile_pool(name="psum", bufs=1, space="PSUM") as psum,
        ):
            # --- gather indices: (b*T + t)*V + id, partition = t.  iota
            # steps must fit int16, so the *V happens on DVE; the ids land
            # on top of the row base via SWDGE accumulate-during-DMA ---
            btb = pool.tile([P, W], i32)
            nc.gpsimd.iota(
                btb[:], pattern=[[T, BL], [0, K]], base=0, channel_multiplier=1
            )
            idx = pool.tile([P, W], i32)
            nc.vector.tensor_scalar_mul(out=idx[:], in0=btb[:], scalar1=V)
            nc.gpsimd.dma_start(
                out=idx[:T, :],
                in_=ids_in[:],
                accum_op=mybir.AluOpType.add,
            )

            # --- per-element gathers ([128, 1] per instruction) + per-user
            # compute, so user 0's ACT chain overlaps user 1's gathers ---
            vals = pool.tile([P, W], f32)
            nc.vector.memset(vals[:], 0.0)
            # one shared bounds register instead of a RegisterMove per gather
            bc_reg = nc.gpsimd.to_reg(DLEN - 1)
            d = pool.tile([P, BL * S], f32)
            sg = pool.tile([P, BL * S], f32)
            m2 = pool.tile([P, BL * 2], f32)
            prod = pool.tile([P, BL], f32)
            for b in range(BL):
                for c in range(b * K, (b + 1) * K):
                    nc.gpsimd.indirect_dma_start(
                        out=vals[:, c : c + 1],
                        out_offset=None,
                        in_=data[:, None],
                        in_offset=bass.IndirectOffsetOnAxis(
                            ap=idx[:, c : c + 1], axis=0
                        ),
                        bounds_check=bc_reg,
                        oob_is_err=False,
                    )
                bs = slice(b * S, (b + 1) * S)
                nc.vector.tensor_tensor(
                    out=d[:, bs],
                    in0=vals[:, b * K : b * K + 1].to_broadcast([P, S]),
                    in1=vals[:, b * K + 1 : (b + 1) * K],
                    op=mybir.AluOpType.subtract,
                )
                # sigmoid per user so the first one hides under the other
                # user's gathers (and only one sigmoid-table load total)
                nc.scalar.activation(
                    out=sg[:, bs],
                    in_=d[:, bs],
                    func=mybir.ActivationFunctionType.Sigmoid,
                )
                # sum_s ln(sig) == ln(prod_s sig): pair-multiply on DVE so a
                # single Ln instruction (one ln-table load) finishes the chain
                nc.vector.tensor_tensor(
                    out=m2[:, 2 * b : 2 * b + 2],
                    in0=sg[:, b * S : b * S + 2],
                    in1=sg[:, b * S + 2 : b * S + 4],
                    op=mybir.AluOpType.mult,
                )
                nc.vector.tensor_tensor(
                    out=prod[:, b : b + 1],
                    in0=m2[:, 2 * b : 2 * b + 1],
                    in1=m2[:, 2 * b + 1 : 2 * b + 2],
                    op=mybir.AluOpType.mult,
                )
            # --- x_lens broadcast: one step-0 DMA replicates the two f32
            # values at data[FLAT + b*V] to every partition ---
            xlf = pool.tile([P, BL], f32)
            nc.gpsimd.dma_start(
                out=xlf[:],
                in_=data[FLAT : FLAT + V + 1 : V][None, :].to_broadcast([P, BL]),
            )
            inv = pool.tile([P, BL], f32)
            nc.vector.reciprocal(out=inv[:], in_=xlf[:])
            tio = pool.tile([P, 1], i32)
            nc.gpsimd.iota(tio[:], pattern=[[0, 1]], base=0, channel_multiplier=1)
            tf = pool.tile([P, 1], f32)
            nc.vector.tensor_copy(out=tf[:], in_=tio[:])
            mask = pool.tile([P, BL], f32)
            nc.vector.tensor_tensor(
                out=mask[:],
                in0=tf[:].to_broadcast([P, BL]),
                in1=xlf[:],
                op=mybir.AluOpType.is_lt,
            )
            w = pool.tile([P, BL], f32)
            nc.vector.tensor_tensor(
                out=w[:], in0=mask[:], in1=inv[:], op=mybir.AluOpType.mult
            )

            spsum = pool.tile([P, BL], f32)
            nc.scalar.activation(
                out=spsum[:], in_=prod[:], func=mybir.ActivationFunctionType.Ln
            )

            # stage through DVE so the mult needs only one cross-engine wait
            spsum2 = pool.tile([P, BL], f32)
            nc.vector.tensor_copy(out=spsum2[:], in_=spsum[:])
            sc = pool.tile([P, BL], f32)
            nc.vector.tensor_tensor(
                out=sc[:], in0=spsum2[:], in1=w[:], op=mybir.AluOpType.mult
            )

            # --- partition-axis sum via ones matmul on PE, scale by -1/B ---
            ones = pool.tile([P, 1], f32)
            nc.vector.memset(ones[:], 1.0)
            acc = psum.tile([1, BL], f32, space="PSUM")
            nc.tensor.matmul(
                out=acc[:], lhsT=ones[:], rhs=sc[:], start=True, stop=True
            )
            res = pool.tile([1, 1], f32)
            nc.vector.reduce_sum(out=res[:], in_=acc[:], axis=mybir.AxisListType.X)
            res2 = pool.tile([1, 1], f32)
            nc.scalar.mul(out=res2[:], in_=res[:], mul=-1.0 / B)
            nc.sync.dma_start(out=partial[:], in_=res2[:])

    nc.compile()
    return nc


def make_in_maps(output, labels, x_lens, neg_ids):
    out_np = np.asarray(output, dtype=np.float32)
    lab_np = np.asarray(labels, dtype=np.int32)
    neg_np = np.asarray(neg_ids, dtype=np.int32)
    xl_np = np.asarray(x_lens, dtype=np.int32)
    # [B, T, K] packed per core as [T, BL*K] so the device load is one
    # fully contiguous DMA (40B per partition row)
    ids_np = np.concatenate([lab_np[:, :, None], neg_np], axis=2)
    in_maps = []
    for i in range(NCORES):
        sl = slice(i * BL, (i + 1) * BL)
        data = np.empty(DLEN, dtype=np.float32)
        data[:FLAT] = out_np[sl].reshape(-1)
        data[FLAT:] = 0.0
        for b in range(BL):
            data[FLAT + b * V] = np.float32(xl_np[i * BL + b])
        ids_core = np.ascontiguousarray(
            ids_np[sl].transpose(1, 0, 2).reshape(T, W)
        )
        in_maps.append({"data": data, "ids_in": ids_core})
    return in_maps


def kernel(output, labels, x_lens, uids, neg_ids):
    from concourse.bass_utils import run_bass_kernel_spmd

    nc = build_bass()
    in_maps = make_in_maps(output, labels, x_lens, neg_ids)
    results = run_bass_kernel_spmd(nc, in_maps, list(range(NCORES))).results
    partials = np.stack(
        [np.asarray(results[i]["partial"]).reshape(()) for i in range(NCORES)]
    )
    return partials.sum(dtype=np.float32).reshape(1)


# revision 27
# speedup vs baseline: 1.0197x; 1.0003x over previous
"""BPR loss with pre-sampled negatives, data-parallel over batch on 8 NeuronCores.

Reference computation (B=16, T=100, V=50000, S=4):
    pos = output[b, t, labels[b, t]]
    neg = output[b, t, neg_ids[b, t, s]]
    ls  = log_sigmoid(pos - neg)
    loss = mean_b( -sum_t(mask_t * sum_s ls) / x_lens[b] )

Only 5 of the 50000 vocab logits per (b, t) are touched, so each core
gathers its 1000 needed elements with indirect DMAs instead of streaming
its 40MB shard.  Sharding (host) is pure relayout: the batch is split 2
users/core; labels+neg_ids are packed into one [BL, T, 5] index tensor;
x_lens values (as f32) are appended to the flat logit shard at elements
FLAT + b*V.

Per core (2 users):
  - one SWDGE DMA loads the ids transposed to [t, (b k)] layout,
    accumulated on top of an iota-built row base (b*T + t)*V
  - 10 indirect DMAs ([128, 1] each — the HW consumes one gather index
    per partition channel per instruction) fetch the pos/neg logits
  - x_lens broadcasts to all partitions via one step-0 DMA
  - sigmoid(pos - neg) on ACT per user (overlapping the other user's
    gathers), then sum_s ln(sig) = ln(prod_s sig): pair-products on DVE
    and a single Ln — exactly two activation-table loads
  - weights (t < x_lens[b]) / x_lens[b] on DVE
  - partition-axis sum via ones-vector matmul on the otherwise idle PE
  - core writes one scalar partial; host sums the 8 partials.
"""

import os
import sys

import numpy as np

for _p in ("/opt/trn_rl_repo", "/root/.axon_site/_ro/trn_rl_repo"):
    if os.path.isdir(_p) and _p not in sys.path:
        sys.path.insert(0, _p)

import concourse.bass as bass
import concourse.mybir as mybir
from concourse import bacc
from concourse.tile import TileContext

B, T, V, S = 16, 100, 50000, 4
NCORES = 8
BL = B // NCORES          # users per core
K = S + 1                 # ids per (b, t): label + S negs
W = BL * K                # gather columns
P = 128
FLAT = BL * T * V         # flat size of the per-core logit shard
DLEN = FLAT + V + 1       # + tail holding x_lens (f32 values) at FLAT + b*V

f32 = mybir.dt.float32
i32 = mybir.dt.int32


def build_bass() -> bass.Bass:
    nc = bacc.Bacc("TRN2", target_bir_lowering=False, debug=False)
    data = nc.declare_dram_parameter("data", [DLEN], f32, isOutput=False)
    ids_in = nc.declare_dram_parameter("ids_in", [T, W], i32, isOutput=False)
    partial = nc.declare_dram_parameter("partial", [1, 1], f32, isOutput=True)

    with TileContext(nc) as tc:
        with (
            tc.tile_pool(name="pool", bufs=1) as pool,
            tc.tile_pool(name="psum", bufs=1, space="PSUM") as psum,
        ):
            # --- gather indices: (b*T + t)*V + id, partition = t.  iota
            # steps must fit int16, so the *V happens on DVE; the ids land
            # on top of the row base via SWDGE accumulate-during-DMA ---
            btb = pool.tile([P, W], i32)
            nc.gpsimd.iota(
                btb[:], pattern=[[T, BL], [0, K]], base=0, channel_multiplier=1
            )
            idx = pool.tile([P, W], i32)
            nc.vector.tensor_scalar_mul(out=idx[:], in0=btb[:], scalar1=V)
            nc.gpsimd.dma_start(
                out=idx[:T, :],
                in_=ids_in[:],
                accum_op=mybir.AluOpType.add,
            )

            # --- per-element gathers ([128, 1] per instruction) + per-user
            # compute, so user 0's ACT chain overlaps user 1's gathers ---
            vals = pool.tile([P, W], f32)
            nc.vector.memset(vals[:], 0.0)
            # one shared bounds register instead of a RegisterMove per gather
            bc_reg = nc.gpsimd.to_reg(DLEN - 1)
            d = pool.tile([P, BL * S], f32)
            sg = pool.tile([P, BL * S], f32)
            m2 = pool.tile([P, BL * 2], f32)
            prod = pool.tile([P, BL], f32)
            for b in range(BL):
                for c in range(b * K, (b + 1) * K):
                    nc.gpsimd.indirect_dma_start(
                        out=vals[:, c : c + 1],
                        out_offset=None,
                        in_=data[:, None],
                        in_offset=bass.IndirectOffsetOnAxis(
                            ap=idx[:, c : c + 1], axis=0
                        ),
                        bounds_check=bc_reg,
                        oob_is_err=False,
                    )
                bs = slice(b * S, (b + 1) * S)
                nc.vector.tensor_tensor(
                    out=d[:, bs],
                    in0=vals[:, b * K : b * K + 1].to_broadcast([P, S]),
                    in1=vals[:, b * K + 1 : (b + 1) * K],
                    op=mybir.AluOpType.subtract,
                )
                # sigmoid per user so the first one hides under the other
                # user's gathers (and only one sigmoid-table load total)
                nc.scalar.activation(
                    out=sg[:, bs],
                    in_=d[:, bs],
                    func=mybir.ActivationFunctionType.Sigmoid,
                )
                # sum_s ln(sig) == ln(prod_s sig): pair-multiply on DVE so a
                # single Ln instruction (one ln-table load) finishes the chain
                nc.vector.tensor_tensor(
                    out=m2[:, 2 * b : 2 * b + 2],
                    in0=sg[:, b * S : b * S + 2],
                    in1=sg[:, b * S + 2 : b * S + 4],
                    op=mybir.AluOpType.mult,
                )
                nc.vector.tensor_tensor(
                    out=prod[:, b : b + 1],
                    in0=m2[:, 2 * b : 2 * b + 1],
                    in1=m2[:, 2 * b + 1 : 2 * b + 2],
                    op=mybir.AluOpType.mult,
                )
            # --- x_lens broadcast: one step-0 DMA replicates the two f32
            # values at data[FLAT + b*V] to every partition ---
            xlf = pool.tile([P, BL], f32)
            nc.gpsimd.dma_start(
                out=xlf[:],
                in_=data[FLAT : FLAT + V + 1 : V][None, :].to_broadcast([P, BL]),
            )
            inv = pool.tile([P, BL], f32)
            nc.vector.reciprocal(out=inv[:], in_=xlf[:])
            # btb column 0 is already the partition index t (b=0, k=0)
            tf = pool.tile([P, 1], f32)
            nc.vector.tensor_copy(out=tf[:], in_=btb[:, 0:1])
            mask = pool.tile([P, BL], f32)
            nc.vector.tensor_tensor(
                out=mask[:],
                in0=tf[:].to_broadcast([P, BL]),
                in1=xlf[:],
                op=mybir.AluOpType.is_lt,
            )
            w = pool.tile([P, BL], f32)
            nc.vector.tensor_tensor(
                out=w[:], in0=mask[:], in1=inv[:], op=mybir.AluOpType.mult
            )

            spsum = pool.tile([P, BL], f32)
            nc.scalar.activation(
                out=spsum[:], in_=prod[:], func=mybir.ActivationFunctionType.Ln
            )

            sc = pool.tile([P, BL], f32)
            nc.vector.tensor_tensor(
                out=sc[:], in0=spsum[:], in1=w[:], op=mybir.AluOpType.mult
            )

            # --- partition-axis sum via ones matmul on PE, scale by -1/B ---
            ones = pool.tile([P, 1], f32)
            nc.vector.memset(ones[:], 1.0)
            acc = psum.tile([1, BL], f32, space="PSUM")
            nc.tensor.matmul(
                out=acc[:], lhsT=ones[:], rhs=sc[:], start=True, stop=True
            )
            res = pool.tile([1, 1], f32)
            nc.vector.reduce_sum(out=res[:], in_=acc[:], axis=mybir.AxisListType.X)
            res2 = pool.tile([1, 1], f32)
            nc.scalar.mul(out=res2[:], in_=res[:], mul=-1.0 / B)
            nc.sync.dma_start(out=partial[:], in_=res2[:])

    nc.compile()
    return nc


def make_in_maps(output, labels, x_lens, neg_ids):
    out_np = np.asarray(output, dtype=np.float32)
    lab_np = np.asarray(labels, dtype=np.int32)
    neg_np = np.asarray(neg_ids, dtype=np.int32)
    xl_np = np.asarray(x_lens, dtype=np.int32)
    # [B, T, K] packed per core as [T, BL*K] so the device load is one
    # fully contiguous DMA (40B per partition row)
    ids_np = np.concatenate([lab_np[:, :, None], neg_np], axis=2)
    in_maps = []
    for i in range(NCORES):
        sl = slice(i * BL, (i + 1) * BL)
        data = np.empty(DLEN, dtype=np.float32)
        data[:FLAT] = out_np[sl].reshape(-1)
        data[FLAT:] = 0.0
        for b in range(BL):
            data[FLAT + b * V] = np.float32(xl_np[i * BL + b])
        ids_core = np.ascontiguousarray(
            ids_np[sl].transpose(1, 0, 2).reshape(T, W)
        )
        in_maps.append({"data": data, "ids_in": ids_core})
    return in_maps


def kernel(output, labels, x_lens, uids, neg_ids):
    from concourse.bass_utils import run_bass_kernel_spmd

    nc = build_bass()
    in_maps = make_in_maps(output, labels, x_lens, neg_ids)
    results = run_bass_kernel_spmd(nc, in_maps, list(range(NCORES))).results
    partials = np.stack(
        [np.asarray(results[i]["partial"]).reshape(()) for i in range(NCORES)]
    )
    return partials.sum(dtype=np.float32).reshape(1)
